# revision 9
# baseline (speedup 1.0000x reference)
"""DeepSpeed-style fused MLP (residual-add + LayerNorm + GEMM1 + GELU + GEMM2
+ bias/residual add) on 8 Trainium2 NeuronCores.

Strategy (data-parallel over tokens — no collectives):
  - Each core owns T/8 = 512 tokens end-to-end. attn_nw/attn_nb are folded
    into inter_w/inter_b on the host; the attention-output bias is folded
    into the input (xin = x + bias), output_b into the final-add residual
    (xresf = residual + output_b).
  - Per core: LayerNorm its 512 tokens in fp32, PE-transpose the normalized
    activations to lnT [H, 512] bf16 (SBUF-resident), GEMM1 (bf16, fp32
    accum) streaming full inter_w -> bias+gelu on ScalarE -> interT
    [I, 512] bf16 (SBUF-resident) -> GEMM2 streaming full output_w ->
    psum + xin + xresf -> write its disjoint token slice.
  - Host concatenates the 8 disjoint token slices.

Self-contained: hardcodes the problem shapes (B=2, S=2048, H=4096, I=16384).
"""
import numpy as np
import ml_dtypes

BF16_NP = ml_dtypes.bfloat16

# problem shapes
B, S, H, I = 2, 2048, 4096, 16384
T = B * S
NC = 8
P = 128
TPC = T // NC            # tokens per core
W2C = 2                  # i-tiles per GEMM2 weight chunk
FN = 512                 # GEMM2 h-strip width
EPS = 1e-12

_BUILD_CACHE = {}


def _build(tpc=TPC, h=H, i_dim=I, reps=1, nd=NC):
    import concourse.mybir as mybir
    import concourse.tile as tile
    from concourse import bacc
    from concourse.bass import ts
    from concourse.masks import make_identity
    from contextlib import ExitStack

    f32 = mybir.dt.float32
    bf16 = mybir.dt.bfloat16
    A = mybir.ActivationFunctionType
    OP = mybir.AluOpType

    hc = h // P
    icnt = i_dim // P
    w2c = min(W2C, icnt)
    icc_cnt = icnt // w2c
    fn = min(FN, h)
    hn_cnt = h // fn
    ntb = tpc // P

    nc = bacc.Bacc(trn_type="TRN2", num_devices=nd)

    # xin = x + attn-output bias (host-folded); xresf = residual + output_b
    xin = nc.dram_tensor("xin", (tpc, h), f32, kind="ExternalInput")
    xres = nc.dram_tensor("xres", (tpc, h), f32, kind="ExternalInput")
    xresf = nc.dram_tensor("xresf", (tpc, h), f32, kind="ExternalInput")
    w1 = nc.dram_tensor("w1", (icnt, P, hc * P), bf16, kind="ExternalInput")
    bi = nc.dram_tensor("bi", (P, icnt), f32, kind="ExternalInput")
    w2 = nc.dram_tensor("w2", (hn_cnt * icc_cnt, P, w2c * fn), bf16,
                        kind="ExternalInput")
    out_ext = nc.dram_tensor("out", (tpc, h), f32, kind="ExternalOutput")

    with tile.TileContext(nc) as tc, ExitStack() as ctx:
        consts = ctx.enter_context(tc.tile_pool(name="consts", bufs=1))
        big = ctx.enter_context(tc.tile_pool(name="big", bufs=1))
        ident = consts.tile([P, P], bf16)
        make_identity(nc, ident[:])
        eps_t = consts.tile([P, 1], f32)
        nc.vector.memset(eps_t[:], EPS)
        bi_sb = consts.tile([P, icnt], f32)
        nc.sync.dma_start(bi_sb[:], bi[:])

        for rep in range(reps):
            lnT = big.tile([P, hc, tpc], bf16, tag="lnT")
            interT = big.tile([P, icnt, tpc], bf16, tag="interT")

            # Region-A pools (live through GEMM2) open first so the nested
            # LN/GEMM1 pools reuse a disjoint SBUF region: next rep's LN can
            # then run during this rep's GEMM2 (it only waits on w1p/lnp of
            # the PREVIOUS rep, both dead by GEMM1's end).
            with tc.tile_pool(name="w2p", bufs=3) as w2p, \
                 tc.tile_pool(name="fap", bufs=2) as fap, \
                 tc.tile_pool(name="ps2", bufs=1, space="PSUM") as ps2p:
                # ---- Stage 1: LayerNorm + PE transpose into lnT ----
                with tc.tile_pool(name="lnp", bufs=1) as lnp, \
                     tc.tile_pool(name="lnbp", bufs=1) as lnbp, \
                     tc.tile_pool(name="rap", bufs=2) as rap, \
                     tc.tile_pool(name="stat", bufs=2) as stat, \
                     tc.tile_pool(name="pstr", bufs=2, space="PSUM") as pstr:
                    hh = h // 2
                    for tb in range(ntb):
                        lnb = lnbp.tile([P, h], bf16, tag="lnb")
                        nmean = stat.tile([P, 1], f32, tag="nmean")
                        ssq = stat.tile([P, 1], f32, tag="ssq")
                        rahs = []
                        for hv in range(2):
                            rah = rap.tile([P, hh], f32, tag="rah")
                            nc.gpsimd.dma_start(rah[:],
                                                xin[ts(tb, P), ts(hv, hh)])
                            rh = lnp.tile([P, hh], f32, tag="rh")
                            nc.gpsimd.dma_start(rh[:],
                                                xres[ts(tb, P), ts(hv, hh)])
                            nc.vector.tensor_add(rah[:], rah[:], rh[:])
                            rahs.append(rah)
                            # partial stats per half: sum into nmean/ssq via
                            # accumulating reduce (second half adds)
                            nm_h = stat.tile([P, 1], f32, tag=f"nm{hv}",
                                             name=f"nm{hv}")
                            nc.vector.reduce_sum(nm_h[:], rah[:],
                                                 axis=mybir.AxisListType.X)
                            sq_h = stat.tile([P, 1], f32, tag=f"sq{hv}",
                                             name=f"sq{hv}")
                            nc.scalar.activation(lnb[:, ts(hv, hh)], rah[:],
                                                 A.Square, accum_out=sq_h[:])
                            if hv == 0:
                                nm0, sq0 = nm_h, sq_h
                            else:
                                nc.vector.tensor_add(nmean[:], nm0[:], nm_h[:])
                                nc.vector.tensor_add(ssq[:], sq0[:], sq_h[:])
                        nc.scalar.mul(nmean[:], nmean[:], -1.0 / h)
                        var = stat.tile([P, 1], f32, tag="var")
                        nc.vector.tensor_scalar_mul(var[:], ssq[:], 1.0 / h)
                        msq = stat.tile([P, 1], f32, tag="msq")
                        nc.vector.tensor_mul(msq[:], nmean[:], nmean[:])
                        nc.vector.tensor_sub(var[:], var[:], msq[:])
                        rstd = stat.tile([P, 1], f32, tag="rstd")
                        nc.scalar.activation(rstd[:], var[:], A.Sqrt,
                                             bias=eps_t[:])
                        nc.vector.reciprocal(rstd[:], rstd[:])
                        for hv in range(2):
                            nc.vector.tensor_scalar(lnb[:, ts(hv, hh)],
                                                    rahs[hv][:], nmean[:],
                                                    rstd[:],
                                                    op0=OP.add, op1=OP.mult)
                        for hcb in range(hc):
                            ps_tr = pstr.tile([P, P], bf16, tag="ps_tr")
                            nc.tensor.transpose(ps_tr[:], lnb[:, ts(hcb, P)],
                                                ident[:])
                            nc.vector.tensor_copy(lnT[:, hcb, ts(tb, P)],
                                                  ps_tr[:])

                # ---- Stage 2: GEMM1 -> gelu -> interT (w1p reuses LN region)
                with tc.tile_pool(name="w1p", bufs=3) as w1p, \
                     tc.tile_pool(name="ps1", bufs=3, space="PSUM") as ps1p:
                    for ic in range(icnt):
                        w1t = w1p.tile([P, hc * P], bf16, tag="w1t")
                        nc.sync.dma_start(w1t[:], w1[ic])
                        ps = ps1p.tile([P, tpc], f32, tag="ps")
                        for hcb in range(hc):
                            nc.tensor.matmul(ps[:], w1t[:, ts(hcb, P)],
                                             lnT[:, hcb, :],
                                             start=(hcb == 0),
                                             stop=(hcb == hc - 1))
                        nc.scalar.activation(interT[:, ic, :], ps[:],
                                             A.Gelu_apprx_tanh,
                                             bias=bi_sb[:, ic:ic + 1])

                # ---- Stage 3: GEMM2 -> psum + xin + xresf -> out ----
                for hn in range(hn_cnt):
                    pss = [ps2p.tile([P, fn], f32, tag=f"ps2_{tsb}",
                                     name=f"ps2_{tsb}")
                           for tsb in range(ntb)]
                    for icc in range(icc_cnt):
                        w2t = w2p.tile([P, w2c * fn], bf16, tag="w2t")
                        nc.scalar.dma_start(w2t[:], w2[hn * icc_cnt + icc])
                        for tsb in range(ntb):
                            for j in range(w2c):
                                icg = icc * w2c + j
                                nc.tensor.matmul(
                                    pss[tsb][:], interT[:, icg, ts(tsb, P)],
                                    w2t[:, ts(j, fn)],
                                    start=(icc == 0 and j == 0),
                                    stop=(icc == icc_cnt - 1 and
                                          j == w2c - 1))
                    for tsb in range(ntb):
                        fx = fap.tile([P, fn], f32, tag="fx")
                        nc.sync.dma_start(fx[:], xin[ts(tsb, P), ts(hn, fn)])
                        fr = fap.tile([P, fn], f32, tag="fr")
                        nc.sync.dma_start(fr[:], xresf[ts(tsb, P), ts(hn, fn)])
                        nc.vector.tensor_add(fx[:], fx[:], pss[tsb][:])
                        nc.vector.tensor_add(fx[:], fx[:], fr[:])
                        nc.sync.dma_start(out_ext[ts(tsb, P), ts(hn, fn)],
                                          fx[:])
    nc.finalize()
    return nc


def get_nc(tpc=TPC, h=H, i_dim=I, reps=1, nd=NC):
    key = (tpc, h, i_dim, reps, nd)
    if key not in _BUILD_CACHE:
        _BUILD_CACHE[key] = _build(tpc, h, i_dim, reps, nd)
    return _BUILD_CACHE[key]


def prep_weights(bias, attn_nw, attn_nb, inter_w, inter_b, output_w, output_b,
                 h=H, i_dim=I):
    """Host-side weight folding + tiling. Returns dict of shared tensors."""
    nw = np.asarray(attn_nw, dtype=np.float32)
    nb = np.asarray(attn_nb, dtype=np.float32)
    wi = np.asarray(inter_w, dtype=np.float32)
    ib = np.asarray(inter_b, dtype=np.float32)
    wo = np.asarray(output_w, dtype=np.float32)

    hc = h // P
    icnt = i_dim // P
    w2c = min(W2C, icnt)
    icc_cnt = icnt // w2c
    fn = min(FN, h)
    hn_cnt = h // fn

    w_eff = wi * nw[None, :]                     # [I, H]
    w1_host = np.ascontiguousarray(
        w_eff.reshape(icnt, P, hc, P).transpose(0, 3, 2, 1)
        .reshape(icnt, P, hc * P)).astype(BF16_NP)
    bi_eff = ib + wi @ nb                        # [I]
    bi_host = np.ascontiguousarray(bi_eff.reshape(icnt, P).T)
    w2_host = np.ascontiguousarray(
        wo.T.reshape(icc_cnt, w2c, P, hn_cnt, fn).transpose(3, 0, 2, 1, 4)
        .reshape(hn_cnt * icc_cnt, P, w2c * fn)).astype(BF16_NP)
    return {"w1": w1_host, "bi": bi_host, "w2": w2_host}


def prep_in_maps(input, residual, bias, attn_nw, attn_nb, inter_w, inter_b,
                 output_w, output_b):
    x2 = np.asarray(input, dtype=np.float32).reshape(T, H)
    r2 = np.asarray(residual, dtype=np.float32).reshape(T, H)
    bias = np.asarray(bias, dtype=np.float32)
    ob = np.asarray(output_b, dtype=np.float32)

    wts = prep_weights(bias, attn_nw, attn_nb, inter_w, inter_b,
                       output_w, output_b)
    xin_full = x2 + bias[None, :]
    xresf_full = r2 + ob[None, :]

    in_maps = []
    for c in range(NC):
        sl = slice(c * TPC, (c + 1) * TPC)
        in_maps.append({
            "xin": np.ascontiguousarray(xin_full[sl]),
            "xres": np.ascontiguousarray(r2[sl]),
            "xresf": np.ascontiguousarray(xresf_full[sl]),
            **wts,
        })
    return in_maps


def assemble(results):
    return np.concatenate([r["out"] for r in results], axis=0)


def run(inputs, trace=False):
    from concourse import bass_utils
    nc = get_nc()
    in_maps = prep_in_maps(**inputs)
    res = bass_utils.run_bass_kernel_spmd(
        nc, in_maps, core_ids=list(range(NC)), trace=trace)
    return assemble(res.results), res


def kernel(**inputs):
    out, _ = run(inputs)
    return out.reshape(B, S, H).astype(np.float32)


# revision 11
# speedup vs baseline: 1.0451x; 1.0451x over previous
"""DeepSpeed-style fused MLP (residual-add + LayerNorm + GEMM1 + GELU + GEMM2
+ bias/residual add) on 8 Trainium2 NeuronCores.

Strategy (data-parallel over tokens — no collectives):
  - Each core owns T/8 = 512 tokens end-to-end. attn_nw/attn_nb are folded
    into inter_w/inter_b on the host; the attention-output bias is folded
    into the input (xin = x + bias), output_b into the final-add residual
    (xresf = residual + output_b).
  - Per core: LayerNorm its 512 tokens in fp32, PE-transpose the normalized
    activations to lnT [H, 512] bf16 (SBUF-resident), GEMM1 (bf16, fp32
    accum) streaming full inter_w -> bias+gelu on ScalarE -> interT
    [I, 512] bf16 (SBUF-resident) -> GEMM2 streaming full output_w ->
    psum + xin + xresf -> write its disjoint token slice.
  - Host concatenates the 8 disjoint token slices.

Self-contained: hardcodes the problem shapes (B=2, S=2048, H=4096, I=16384).
"""
import numpy as np
import ml_dtypes

BF16_NP = ml_dtypes.bfloat16

# problem shapes
B, S, H, I = 2, 2048, 4096, 16384
T = B * S
NC = 8
P = 128
TPC = T // NC            # tokens per core
W2C = 8                  # i-tiles per GEMM2 weight chunk
FN = 512                 # GEMM2 h-strip width
EPS = 1e-12

_BUILD_CACHE = {}


def _build(tpc=TPC, h=H, i_dim=I, reps=1, nd=NC):
    import concourse.mybir as mybir
    import concourse.tile as tile
    from concourse import bacc
    from concourse.bass import ts
    from concourse.masks import make_identity
    from contextlib import ExitStack

    f32 = mybir.dt.float32
    bf16 = mybir.dt.bfloat16
    A = mybir.ActivationFunctionType
    OP = mybir.AluOpType

    hc = h // P
    icnt = i_dim // P
    w2c = min(W2C, icnt)
    icc_cnt = icnt // w2c
    fn = min(FN, h)
    hn_cnt = h // fn
    ntb = tpc // P

    nc = bacc.Bacc(trn_type="TRN2", num_devices=nd)

    # xin = x + attn-output bias (host-folded); xresf = residual + output_b
    xin = nc.dram_tensor("xin", (tpc, h), f32, kind="ExternalInput")
    xres = nc.dram_tensor("xres", (tpc, h), f32, kind="ExternalInput")
    xresf = nc.dram_tensor("xresf", (tpc, h), f32, kind="ExternalInput")
    w1 = nc.dram_tensor("w1", (icnt, P, hc * P), bf16, kind="ExternalInput")
    bi = nc.dram_tensor("bi", (P, icnt), f32, kind="ExternalInput")
    w2 = nc.dram_tensor("w2", (hn_cnt * icc_cnt, P, w2c * fn), bf16,
                        kind="ExternalInput")
    out_ext = nc.dram_tensor("out", (tpc, h), f32, kind="ExternalOutput")

    with tile.TileContext(nc) as tc, ExitStack() as ctx:
        consts = ctx.enter_context(tc.tile_pool(name="consts", bufs=1))
        big = ctx.enter_context(tc.tile_pool(name="big", bufs=1))
        ident = consts.tile([P, P], bf16)
        make_identity(nc, ident[:])
        eps_t = consts.tile([P, 1], f32)
        nc.vector.memset(eps_t[:], EPS)
        bi_sb = consts.tile([P, icnt], f32)
        nc.sync.dma_start(bi_sb[:], bi[:])

        for rep in range(reps):
            lnT = big.tile([P, hc, tpc], bf16, tag="lnT")
            interT = big.tile([P, icnt, tpc], bf16, tag="interT")

            # ---- Stage 1: LayerNorm + PE transpose into lnT [h, tpc] ----
            with tc.tile_pool(name="lnp", bufs=1) as lnp, \
                 tc.tile_pool(name="lnbp", bufs=2) as lnbp, \
                 tc.tile_pool(name="stat", bufs=2) as stat, \
                 tc.tile_pool(name="pstr", bufs=2, space="PSUM") as pstr:
                hh = h // 2
                for tb in range(ntb):
                    ra = lnp.tile([P, h], f32, tag="ra")
                    nc.sync.dma_start(ra[:], xin[ts(tb, P)])
                    lnb = lnbp.tile([P, h], bf16, tag="lnb")
                    for hv in range(2):
                        rh = lnp.tile([P, hh], f32, tag="rh")
                        nc.sync.dma_start(rh[:], xres[ts(tb, P), ts(hv, hh)])
                        nc.vector.tensor_add(ra[:, ts(hv, hh)],
                                             ra[:, ts(hv, hh)], rh[:])
                    nmean = stat.tile([P, 1], f32, tag="nmean")
                    nc.vector.reduce_sum(nmean[:], ra[:],
                                         axis=mybir.AxisListType.X)
                    nc.scalar.mul(nmean[:], nmean[:], -1.0 / h)
                    ssq = stat.tile([P, 1], f32, tag="ssq")
                    # lnb doubles as throwaway Square scratch (overwritten
                    # by the real ln values below)
                    nc.scalar.activation(lnb[:], ra[:], A.Square,
                                         accum_out=ssq[:])
                    var = stat.tile([P, 1], f32, tag="var")
                    nc.vector.tensor_scalar_mul(var[:], ssq[:], 1.0 / h)
                    msq = stat.tile([P, 1], f32, tag="msq")
                    nc.vector.tensor_mul(msq[:], nmean[:], nmean[:])
                    nc.vector.tensor_sub(var[:], var[:], msq[:])
                    rstd = stat.tile([P, 1], f32, tag="rstd")
                    nc.scalar.activation(rstd[:], var[:], A.Sqrt,
                                         bias=eps_t[:])
                    nc.vector.reciprocal(rstd[:], rstd[:])
                    nc.vector.tensor_scalar(lnb[:], ra[:], nmean[:], rstd[:],
                                            op0=OP.add, op1=OP.mult)
                    for hcb in range(hc):
                        ps_tr = pstr.tile([P, P], bf16, tag="ps_tr")
                        nc.tensor.transpose(ps_tr[:], lnb[:, ts(hcb, P)],
                                            ident[:])
                        nc.vector.tensor_copy(lnT[:, hcb, ts(tb, P)], ps_tr[:])

            # ---- Stage 2: GEMM1 -> gelu -> interT; GEMM2 -> final add ----
            with tc.tile_pool(name="w1p", bufs=3) as w1p, \
                 tc.tile_pool(name="w2p", bufs=2) as w2p, \
                 tc.tile_pool(name="fap", bufs=2) as fap, \
                 tc.tile_pool(name="frp", bufs=1) as frp, \
                 tc.tile_pool(name="ps1", bufs=2, space="PSUM") as ps1p, \
                 tc.tile_pool(name="ps2", bufs=1, space="PSUM") as ps2p:
                hw1 = hc // 2
                for ic in range(icnt):
                    # w1 row split in two half-tiles for deeper prefetch
                    w1a = w1p.tile([P, hw1 * P], bf16, tag="w1a")
                    nc.sync.dma_start(w1a[:], w1[ic, :, :hw1 * P])
                    w1b = w1p.tile([P, hw1 * P], bf16, tag="w1b")
                    nc.sync.dma_start(w1b[:], w1[ic, :, hw1 * P:])
                    ps = ps1p.tile([P, tpc], f32, tag="ps")
                    for hcb in range(hc):
                        wt = w1a if hcb < hw1 else w1b
                        nc.tensor.matmul(ps[:], wt[:, ts(hcb % hw1, P)],
                                         lnT[:, hcb, :],
                                         start=(hcb == 0),
                                         stop=(hcb == hc - 1))
                    nc.scalar.activation(interT[:, ic, :], ps[:],
                                         A.Gelu_apprx_tanh,
                                         bias=bi_sb[:, ic:ic + 1])
                for hn in range(hn_cnt):
                    pss = [ps2p.tile([P, fn], f32, tag=f"ps2_{tsb}",
                                     name=f"ps2_{tsb}")
                           for tsb in range(ntb)]
                    for icc in range(icc_cnt):
                        w2t = w2p.tile([P, w2c * fn], bf16, tag="w2t")
                        # ACT-ring DMA: keeps weight stream off the SP ring
                        # that carries the final-add loads/stores
                        nc.scalar.dma_start(w2t[:], w2[hn * icc_cnt + icc])
                        for tsb in range(ntb):
                            for j in range(w2c):
                                icg = icc * w2c + j
                                nc.tensor.matmul(
                                    pss[tsb][:], interT[:, icg, ts(tsb, P)],
                                    w2t[:, ts(j, fn)],
                                    start=(icc == 0 and j == 0),
                                    stop=(icc == icc_cnt - 1 and
                                          j == w2c - 1))
                    for tsb in range(ntb):
                        fx = fap.tile([P, fn], f32, tag="fx")
                        nc.sync.dma_start(fx[:], xin[ts(tsb, P), ts(hn, fn)])
                        fr = frp.tile([P, fn], f32, tag="fr")
                        nc.sync.dma_start(fr[:], xresf[ts(tsb, P), ts(hn, fn)])
                        nc.vector.tensor_add(fx[:], fx[:], pss[tsb][:])
                        nc.vector.tensor_add(fx[:], fx[:], fr[:])
                        nc.sync.dma_start(out_ext[ts(tsb, P), ts(hn, fn)],
                                          fx[:])
    nc.finalize()
    return nc


def get_nc(tpc=TPC, h=H, i_dim=I, reps=1, nd=NC):
    key = (tpc, h, i_dim, reps, nd)
    if key not in _BUILD_CACHE:
        _BUILD_CACHE[key] = _build(tpc, h, i_dim, reps, nd)
    return _BUILD_CACHE[key]


def prep_weights(bias, attn_nw, attn_nb, inter_w, inter_b, output_w, output_b,
                 h=H, i_dim=I):
    """Host-side weight folding + tiling. Returns dict of shared tensors."""
    nw = np.asarray(attn_nw, dtype=np.float32)
    nb = np.asarray(attn_nb, dtype=np.float32)
    wi = np.asarray(inter_w, dtype=np.float32)
    ib = np.asarray(inter_b, dtype=np.float32)
    wo = np.asarray(output_w, dtype=np.float32)

    hc = h // P
    icnt = i_dim // P
    w2c = min(W2C, icnt)
    icc_cnt = icnt // w2c
    fn = min(FN, h)
    hn_cnt = h // fn

    w_eff = wi * nw[None, :]                     # [I, H]
    w1_host = np.ascontiguousarray(
        w_eff.reshape(icnt, P, hc, P).transpose(0, 3, 2, 1)
        .reshape(icnt, P, hc * P)).astype(BF16_NP)
    bi_eff = ib + wi @ nb                        # [I]
    bi_host = np.ascontiguousarray(bi_eff.reshape(icnt, P).T)
    w2_host = np.ascontiguousarray(
        wo.T.reshape(icc_cnt, w2c, P, hn_cnt, fn).transpose(3, 0, 2, 1, 4)
        .reshape(hn_cnt * icc_cnt, P, w2c * fn)).astype(BF16_NP)
    return {"w1": w1_host, "bi": bi_host, "w2": w2_host}


def prep_in_maps(input, residual, bias, attn_nw, attn_nb, inter_w, inter_b,
                 output_w, output_b):
    x2 = np.asarray(input, dtype=np.float32).reshape(T, H)
    r2 = np.asarray(residual, dtype=np.float32).reshape(T, H)
    bias = np.asarray(bias, dtype=np.float32)
    ob = np.asarray(output_b, dtype=np.float32)

    wts = prep_weights(bias, attn_nw, attn_nb, inter_w, inter_b,
                       output_w, output_b)
    xin_full = x2 + bias[None, :]
    xresf_full = r2 + ob[None, :]

    in_maps = []
    for c in range(NC):
        sl = slice(c * TPC, (c + 1) * TPC)
        in_maps.append({
            "xin": np.ascontiguousarray(xin_full[sl]),
            "xres": np.ascontiguousarray(r2[sl]),
            "xresf": np.ascontiguousarray(xresf_full[sl]),
            **wts,
        })
    return in_maps


def assemble(results):
    return np.concatenate([r["out"] for r in results], axis=0)


def run(inputs, trace=False):
    from concourse import bass_utils
    nc = get_nc()
    in_maps = prep_in_maps(**inputs)
    res = bass_utils.run_bass_kernel_spmd(
        nc, in_maps, core_ids=list(range(NC)), trace=trace)
    return assemble(res.results), res


def kernel(**inputs):
    out, _ = run(inputs)
    return out.reshape(B, S, H).astype(np.float32)


# revision 12
# speedup vs baseline: 1.0632x; 1.0174x over previous
"""DeepSpeed-style fused MLP (residual-add + LayerNorm + GEMM1 + GELU + GEMM2
+ bias/residual add) on 8 Trainium2 NeuronCores.

Strategy (data-parallel over tokens — no collectives):
  - Each core owns T/8 = 512 tokens end-to-end. attn_nw/attn_nb are folded
    into inter_w/inter_b on the host; the attention-output bias is folded
    into the input (xin = x + bias), output_b into the final-add residual
    (xresf = residual + output_b).
  - Per core: LayerNorm its 512 tokens in fp32, PE-transpose the normalized
    activations to lnT [H, 512] bf16 (SBUF-resident), GEMM1 (bf16, fp32
    accum) streaming full inter_w -> bias+gelu on ScalarE -> interT
    [I, 512] bf16 (SBUF-resident) -> GEMM2 streaming full output_w ->
    psum + xin + xresf -> write its disjoint token slice.
  - Host concatenates the 8 disjoint token slices.

Self-contained: hardcodes the problem shapes (B=2, S=2048, H=4096, I=16384).
"""
import numpy as np
import ml_dtypes

BF16_NP = ml_dtypes.bfloat16

# problem shapes
B, S, H, I = 2, 2048, 4096, 16384
T = B * S
NC = 8
P = 128
TPC = T // NC            # tokens per core
W2C = 8                  # i-tiles per GEMM2 weight chunk
FN = 512                 # GEMM2 h-strip width
EPS = 1e-12

_BUILD_CACHE = {}


def _build(tpc=TPC, h=H, i_dim=I, reps=1, nd=NC):
    import concourse.mybir as mybir
    import concourse.tile as tile
    from concourse import bacc
    from concourse.bass import ts
    from concourse.masks import make_identity
    from contextlib import ExitStack

    f32 = mybir.dt.float32
    bf16 = mybir.dt.bfloat16
    A = mybir.ActivationFunctionType
    OP = mybir.AluOpType

    hc = h // P
    icnt = i_dim // P
    w2c = min(W2C, icnt)
    icc_cnt = icnt // w2c
    fn = min(FN, h)
    hn_cnt = h // fn
    ntb = tpc // P

    nc = bacc.Bacc(trn_type="TRN2", num_devices=nd)

    # xin = x + attn-output bias (host-folded); xresf = residual + output_b
    xin = nc.dram_tensor("xin", (tpc, h), f32, kind="ExternalInput")
    xres = nc.dram_tensor("xres", (tpc, h), f32, kind="ExternalInput")
    xresf = nc.dram_tensor("xresf", (tpc, h), f32, kind="ExternalInput")
    w1 = nc.dram_tensor("w1", (icnt, P, hc * P), bf16, kind="ExternalInput")
    bi = nc.dram_tensor("bi", (P, icnt), f32, kind="ExternalInput")
    w2 = nc.dram_tensor("w2", (hn_cnt * icc_cnt, P, w2c * fn), bf16,
                        kind="ExternalInput")
    out_ext = nc.dram_tensor("out", (tpc, h), f32, kind="ExternalOutput")

    with tile.TileContext(nc) as tc, ExitStack() as ctx:
        consts = ctx.enter_context(tc.tile_pool(name="consts", bufs=1))
        big = ctx.enter_context(tc.tile_pool(name="big", bufs=1))
        ident = consts.tile([P, P], bf16)
        make_identity(nc, ident[:])
        eps_t = consts.tile([P, 1], f32)
        nc.vector.memset(eps_t[:], EPS)
        bi_sb = consts.tile([P, icnt], f32)
        nc.sync.dma_start(bi_sb[:], bi[:])

        for rep in range(reps):
            lnT = big.tile([P, hc, tpc], bf16, tag="lnT")
            interT = big.tile([P, icnt, tpc], bf16, tag="interT")

            # ---- Stage 1: LayerNorm + PE transpose into lnT [h, tpc] ----
            with tc.tile_pool(name="lnp", bufs=1) as lnp, \
                 tc.tile_pool(name="lnbp", bufs=2) as lnbp, \
                 tc.tile_pool(name="stat", bufs=2) as stat, \
                 tc.tile_pool(name="pstr", bufs=2, space="PSUM") as pstr:
                hh = h // 2
                for tb in range(ntb):
                    ra = lnp.tile([P, h], f32, tag="ra")
                    nc.sync.dma_start(ra[:], xin[ts(tb, P)])
                    lnb = lnbp.tile([P, h], bf16, tag="lnb")
                    for hv in range(2):
                        rh = lnp.tile([P, hh], f32, tag="rh")
                        nc.sync.dma_start(rh[:], xres[ts(tb, P), ts(hv, hh)])
                        nc.vector.tensor_add(ra[:, ts(hv, hh)],
                                             ra[:, ts(hv, hh)], rh[:])
                    nmean = stat.tile([P, 1], f32, tag="nmean")
                    nc.vector.reduce_sum(nmean[:], ra[:],
                                         axis=mybir.AxisListType.X)
                    nc.scalar.mul(nmean[:], nmean[:], -1.0 / h)
                    ssq = stat.tile([P, 1], f32, tag="ssq")
                    # lnb doubles as throwaway Square scratch (overwritten
                    # by the real ln values below)
                    nc.scalar.activation(lnb[:], ra[:], A.Square,
                                         accum_out=ssq[:])
                    var = stat.tile([P, 1], f32, tag="var")
                    nc.vector.tensor_scalar_mul(var[:], ssq[:], 1.0 / h)
                    msq = stat.tile([P, 1], f32, tag="msq")
                    nc.vector.tensor_mul(msq[:], nmean[:], nmean[:])
                    nc.vector.tensor_sub(var[:], var[:], msq[:])
                    rstd = stat.tile([P, 1], f32, tag="rstd")
                    nc.scalar.activation(rstd[:], var[:], A.Sqrt,
                                         bias=eps_t[:])
                    nc.vector.reciprocal(rstd[:], rstd[:])
                    nc.vector.tensor_scalar(lnb[:], ra[:], nmean[:], rstd[:],
                                            op0=OP.add, op1=OP.mult)
                    for hcb in range(hc):
                        ps_tr = pstr.tile([P, P], bf16, tag="ps_tr")
                        nc.tensor.transpose(ps_tr[:], lnb[:, ts(hcb, P)],
                                            ident[:])
                        nc.vector.tensor_copy(lnT[:, hcb, ts(tb, P)], ps_tr[:])

            # ---- Stage 2: GEMM1 -> gelu -> interT; GEMM2 -> final add ----
            with tc.tile_pool(name="w1p", bufs=2) as w1p, \
                 tc.tile_pool(name="w2p", bufs=2) as w2p, \
                 tc.tile_pool(name="fap", bufs=3) as fap, \
                 tc.tile_pool(name="ps1", bufs=2, space="PSUM") as ps1p, \
                 tc.tile_pool(name="ps2", bufs=1, space="PSUM") as ps2p:
                for ic in range(icnt):
                    w1t = w1p.tile([P, hc * P], bf16, tag="w1t")
                    nc.sync.dma_start(w1t[:], w1[ic])
                    ps = ps1p.tile([P, tpc], f32, tag="ps")
                    for hcb in range(hc):
                        nc.tensor.matmul(ps[:], w1t[:, ts(hcb, P)],
                                         lnT[:, hcb, :],
                                         start=(hcb == 0),
                                         stop=(hcb == hc - 1))
                    nc.scalar.activation(interT[:, ic, :], ps[:],
                                         A.Gelu_apprx_tanh,
                                         bias=bi_sb[:, ic:ic + 1])
                for hn in range(hn_cnt):
                    pss = [ps2p.tile([P, fn], f32, tag=f"ps2_{tsb}",
                                     name=f"ps2_{tsb}")
                           for tsb in range(ntb)]
                    for icc in range(icc_cnt):
                        w2t = w2p.tile([P, w2c * fn], bf16, tag="w2t")
                        nc.sync.dma_start(w2t[:], w2[hn * icc_cnt + icc])
                        for tsb in range(ntb):
                            for j in range(w2c):
                                icg = icc * w2c + j
                                nc.tensor.matmul(
                                    pss[tsb][:], interT[:, icg, ts(tsb, P)],
                                    w2t[:, ts(j, fn)],
                                    start=(icc == 0 and j == 0),
                                    stop=(icc == icc_cnt - 1 and
                                          j == w2c - 1))
                    for tsb in range(ntb):
                        fx = fap.tile([P, fn], f32, tag="fx")
                        nc.sync.dma_start(fx[:], xin[ts(tsb, P), ts(hn, fn)])
                        fr = fap.tile([P, fn], f32, tag="fr")
                        nc.sync.dma_start(fr[:], xresf[ts(tsb, P), ts(hn, fn)])
                        nc.vector.tensor_add(fx[:], fx[:], pss[tsb][:])
                        nc.vector.tensor_add(fx[:], fx[:], fr[:])
                        nc.sync.dma_start(out_ext[ts(tsb, P), ts(hn, fn)],
                                          fx[:])
    nc.finalize()
    return nc


def get_nc(tpc=TPC, h=H, i_dim=I, reps=1, nd=NC):
    key = (tpc, h, i_dim, reps, nd)
    if key not in _BUILD_CACHE:
        _BUILD_CACHE[key] = _build(tpc, h, i_dim, reps, nd)
    return _BUILD_CACHE[key]


def prep_weights(bias, attn_nw, attn_nb, inter_w, inter_b, output_w, output_b,
                 h=H, i_dim=I):
    """Host-side weight folding + tiling. Returns dict of shared tensors."""
    nw = np.asarray(attn_nw, dtype=np.float32)
    nb = np.asarray(attn_nb, dtype=np.float32)
    wi = np.asarray(inter_w, dtype=np.float32)
    ib = np.asarray(inter_b, dtype=np.float32)
    wo = np.asarray(output_w, dtype=np.float32)

    hc = h // P
    icnt = i_dim // P
    w2c = min(W2C, icnt)
    icc_cnt = icnt // w2c
    fn = min(FN, h)
    hn_cnt = h // fn

    w_eff = wi * nw[None, :]                     # [I, H]
    w1_host = np.ascontiguousarray(
        w_eff.reshape(icnt, P, hc, P).transpose(0, 3, 2, 1)
        .reshape(icnt, P, hc * P)).astype(BF16_NP)
    bi_eff = ib + wi @ nb                        # [I]
    bi_host = np.ascontiguousarray(bi_eff.reshape(icnt, P).T)
    w2_host = np.ascontiguousarray(
        wo.T.reshape(icc_cnt, w2c, P, hn_cnt, fn).transpose(3, 0, 2, 1, 4)
        .reshape(hn_cnt * icc_cnt, P, w2c * fn)).astype(BF16_NP)
    return {"w1": w1_host, "bi": bi_host, "w2": w2_host}


def prep_in_maps(input, residual, bias, attn_nw, attn_nb, inter_w, inter_b,
                 output_w, output_b):
    x2 = np.asarray(input, dtype=np.float32).reshape(T, H)
    r2 = np.asarray(residual, dtype=np.float32).reshape(T, H)
    bias = np.asarray(bias, dtype=np.float32)
    ob = np.asarray(output_b, dtype=np.float32)

    wts = prep_weights(bias, attn_nw, attn_nb, inter_w, inter_b,
                       output_w, output_b)
    xin_full = x2 + bias[None, :]
    xresf_full = r2 + ob[None, :]

    in_maps = []
    for c in range(NC):
        sl = slice(c * TPC, (c + 1) * TPC)
        in_maps.append({
            "xin": np.ascontiguousarray(xin_full[sl]),
            "xres": np.ascontiguousarray(r2[sl]),
            "xresf": np.ascontiguousarray(xresf_full[sl]),
            **wts,
        })
    return in_maps


def assemble(results):
    return np.concatenate([r["out"] for r in results], axis=0)


def run(inputs, trace=False):
    from concourse import bass_utils
    nc = get_nc()
    in_maps = prep_in_maps(**inputs)
    res = bass_utils.run_bass_kernel_spmd(
        nc, in_maps, core_ids=list(range(NC)), trace=trace)
    return assemble(res.results), res


def kernel(**inputs):
    out, _ = run(inputs)
    return out.reshape(B, S, H).astype(np.float32)
